# revision 24
# baseline (speedup 1.0000x reference)
"""Trainium2 Bass kernel for nn_Block_68633577390412.

Pipeline (reference): channel mix (64x64) -> frame into 256 half-overlapping
512-windows -> rfft -> per-(c,k) IIR over frames y_f = (s_f + y_{f-1})*t ->
irfft -> hann window -> overlap-add -> tanh(gain*.)

Sharding: 8 cores = 4 batches x 2 channel-halves, no cross-core traffic.

Per-core design (v2 — hop-block DFT + fp16 datapath):
  - Adjacent frames overlap by half a window (hop 256), so the rfft of frame
    f decomposes into two half-window transforms: S_f = A_f + (-1)^r A_{f+1}
    where A_h[r] = sum_{w<256} x[256h+w] e^{-i 2pi r w/512}. Each hop block's
    A is computed once (K=256 contraction, half the tensor-engine streams of
    the direct K=512 frame DFT); the +/- combine is a single fused
    scalar_tensor_tensor sweep on the vector engine.
  - Packed-real spectral rows are grouped by frequency parity
    (C0=Re-even, C1=Re-odd, C2=[Re_256|Im-even], C3=Im-odd) so (-1)^r is a
    constant sign per 128-row chunk and the combine needs no row masks.
  - The whole spectral datapath runs in fp16 (inputs, DFT bases, scan state
    s/y, transfer t, output): fp16 keeps ~1e-3 relative accuracy (tolerance
    is 2e-2) and unlocks the DVE 2x/4x wide modes plus half-size DMA.
  - The frame recurrence is the DVE TensorTensorScan instruction with fp32
    internal state, reading fp16 s tiles from SBUF.
  - irfft + hann + overlap-add stay fused as two accumulating matmul streams
    per output tile against precomputed E1/E2 = (irfft-basis * hann * gain)
    halves; tanh runs on the scalar engine over both channels at once and
    the output DMA stores fp16 with a (frame, channel, w) interleaved view.
  - All DMA goes through HWDGE (SP-engine issue, ~0.6us shared-device
    overhead) instead of Pool SWDGE (~1us Pool-engine hold each), freeing
    the Pool engine to zero scan-boundary columns.
  - Mix runs as a single fp16 stream (K=64) into PSUM, copied to the
    time-on-partitions fp16 signal buffer by the scalar engine; spectral
    work starts once the first ~17/32 of the signal is mixed and the
    remaining mix groups fill tensor-engine gaps between spectral pairs.
  - TRN2 instructions carry a single sync-wait slot; a post-pass hoists
    excess Tile-assigned waits onto standalone event-semaphore waits.
"""
import numpy as np
import ml_dtypes
from contextlib import ExitStack

import concourse.bass as bass
import concourse.tile as tile
from concourse import mybir
from concourse.bass_utils import run_bass_kernel_spmd

F32 = mybir.dt.float32
F16 = mybir.dt.float16

B, C, T = 4, 64, 65536
WS, STEP, K, NF = 512, 256, 257, 256
NCORES = 8
CH = 32            # channels per core
NCHUNK = 512       # 128-sample time chunks in T
XG = 32            # mix groups (2048 samples each)
NPAIR = CH // 2    # channel pairs per core

SIGMA = [1.0, -1.0, 1.0, -1.0]   # (-1)^r per packed row chunk C0..C3


def _packed_rows():
    """Packed-real row order: (type, r) per packed row, grouped so that
    (-1)^r is constant per 128-row chunk."""
    rows = []
    rows += [("re", r) for r in range(0, 256, 2)]          # C0: Re even
    rows += [("re", r) for r in range(1, 256, 2)]          # C1: Re odd
    rows += [("re", 256)] + [("im", r) for r in range(2, 256, 2)]  # C2
    rows += [("im", r) for r in range(1, 256, 2)]          # C3: Im odd
    assert len(rows) == 512
    return rows


def _build_matrices():
    rows = _packed_rows()
    w = np.arange(WS, dtype=np.float64)
    wh = w[:STEP]
    # D~ [256, 512]: half-window DFT basis, packed/parity-ordered columns.
    D = np.empty((STEP, 512), dtype=np.float64)
    for j, (ty, r) in enumerate(rows):
        if ty == "re":
            D[:, j] = np.cos(2 * np.pi * r * wh / WS)
        else:
            D[:, j] = -np.sin(2 * np.pi * r * wh / WS)
    # G [512, 512]: packed irfft basis rows in the same order.
    G = np.empty((512, WS), dtype=np.float64)
    for j, (ty, r) in enumerate(rows):
        if ty == "re":
            coef = 1.0 / WS if r in (0, 256) else 2.0 / WS
            G[j, :] = coef * np.cos(2 * np.pi * r * w / WS)
        else:
            G[j, :] = -(2.0 / WS) * np.sin(2 * np.pi * r * w / WS)
    hann = 0.5 * (1.0 - np.cos(2.0 * np.pi * w / WS))
    E1 = G[:, :STEP] * hann[None, :STEP]
    E2 = G[:, STEP:] * hann[None, STEP:]
    return D, E1, E2


def _split_excess_waits(nc):
    """TRN2 instructions have a single sync-wait slot. Tile's semaphore
    assignment can attach several waits to one instruction; hoist the extras
    onto standalone event-semaphore wait instructions on the same engine."""
    ctr = 0
    for fn in nc.m.functions:
        for bb in fn.blocks:
            out = []
            changed = False
            for inst in bb.instructions:
                si = inst.sync_info
                ow = list(si.on_wait) if si and si.on_wait else []
                if len(ow) > 1:
                    for wv in ow[:-1]:
                        ev = mybir.InstEventSemaphore(
                            name=f"WSPLIT-{ctr}", ins=[], outs=[])
                        ctr += 1
                        ev.engine = inst.engine
                        evsi = ev.sync_info
                        if evsi is None:
                            ev.sync_info = mybir.SyncInfo(on_wait=[wv],
                                                          on_update=[])
                        else:
                            evsi.on_wait = [wv]
                        out.append(ev)
                    si.on_wait = [ow[-1]]
                    changed = True
                out.append(inst)
            if changed:
                bb.instructions = out
    return ctr


def _build_program(split=True):
    nc = bass.Bass("TRN2", target_bir_lowering=False, debug=False,
                   num_devices=NCORES)
    xin_t = nc.dram_tensor("xin", [C, T], F16, kind="ExternalInput")
    msb_t = nc.dram_tensor("msb", [C, CH], F16, kind="ExternalInput")
    dmat_t = nc.dram_tensor("dmat", [STEP, 512], F16, kind="ExternalInput")
    emat_t = nc.dram_tensor("emat", [512, 512], F16, kind="ExternalInput")
    # t_ext: per channel pair, 4 segments [0, 1, t(c0)*128, 0, 1, t(c1)*128]
    # (the 0/1 columns reset the merged scan's state / load the carry)
    text_t = nc.dram_tensor("text", [128, NPAIR * 1040], F16,
                            kind="ExternalInput")
    out_t = nc.dram_tensor("out", [CH, 256, 256], F16, kind="ExternalOutput")

    xin = xin_t.ap()
    msb, dmat, emat, text = msb_t.ap(), dmat_t.ap(), emat_t.ap(), text_t.ap()

    with tile.TileContext(nc) as tc, ExitStack() as ctx:
        cpool = ctx.enter_context(tc.tile_pool(name="const", bufs=1))
        m_sb = cpool.tile([C, CH], F16, tag="m_sb")
        dm_sb = cpool.tile([128, 1024], F16, tag="dm_sb")
        em_sb = cpool.tile([128, 2048], F16, tag="em_sb")
        te_sb = cpool.tile([128, NPAIR * 1040], F16, tag="te_sb")
        zc = cpool.tile([128, 1], F16, tag="zc")
        nc.vector.memset(zc[:], 0.0)

        # x'' : mixed signal, time-on-partitions. col = 32*chunk + d_local.
        xsq = cpool.tile([128, CH * (NCHUNK + 2)], F16, tag="xsq")
        nc.vector.memset(xsq[:, CH * NCHUNK:], 0.0)

        xpool = ctx.enter_context(tc.tile_pool(name="xin", bufs=4))

        def load_consts():
            # issued after the first x tiles: nothing here is needed until
            # the first DFT ~12us in, and HWDGE issue order gates the input.
            for i in range(2):
                nc.sync.dma_start(dm_sb[:, 512 * i:512 * (i + 1)],
                                  dmat[128 * i:128 * (i + 1), :])
            for i in range(4):
                nc.sync.dma_start(em_sb[:, 512 * i:512 * (i + 1)],
                                  emat[128 * i:128 * (i + 1), :])

        def load_text(j):
            # one slice covers channel pairs 2j, 2j+1
            nc.sync.dma_start(te_sb[:, 2080 * j:2080 * (j + 1)],
                              text[:, 2080 * j:2080 * (j + 1)])
        mixpool = ctx.enter_context(tc.tile_pool(name="mixps", bufs=2,
                                                 space="PSUM"))
        spool = ctx.enter_context(tc.tile_pool(name="aps", bufs=4,
                                               space="PSUM"))
        opool = ctx.enter_context(tc.tile_pool(name="ops", bufs=2,
                                               space="PSUM"))
        apool = ctx.enter_context(tc.tile_pool(name="asb", bufs=8))
        sbpool = ctx.enter_context(tc.tile_pool(name="ssb", bufs=4))
        ypool = ctx.enter_context(tc.tile_pool(name="y", bufs=4))
        outpool = ctx.enter_context(tc.tile_pool(name="outs", bufs=4))
        # scan carry between frame halves; col = 8p + 2*ri + c01
        carry = cpool.tile([128, 8 * NPAIR], F16, tag="carry")

        xtiles = [None] * (XG // 2)

        def load_group(gt):
            xt = xpool.tile([C, 4096], F16, tag="xt")
            nc.sync.dma_start(xt[:], xin[:, 4096 * gt:4096 * (gt + 1)])
            xtiles[gt] = xt

        def mix_group(g):
            xt = xtiles[g // 2]
            base = 2048 * (g % 2)
            mps = mixpool.tile([128, 512], F32, tag="mps")
            for m in range(16):
                lhsT = xt[:, base + 128 * m:base + 128 * (m + 1)]
                nc.tensor.matmul(mps[:, CH * m:CH * (m + 1)], lhsT, m_sb[:],
                                 start=True, stop=True)
            nc.scalar.copy(xsq[:, 512 * g:512 * (g + 1)], mps[:])

        def spectral_front(p, half, act_copies=2):
            b0 = 128 * half            # first hop block of this half
            # s_ext: 4 segments of 260 cols: [K, L, s(c0)*128, K, L, s(c1)*128]
            # K has t=0 (kills the merged scan's state across segments), L has
            # t=1 with s = the carry value, so one scan instruction covers all
            # 8 (c01, ri) recurrences and the L columns double as the y_{-1}
            # boundary column that the E2 overlap-add stream reads.
            s_ext = sbpool.tile([128, 1040], F16, tag="s")
            if half == 0:
                # zero all K and L columns (carry-in is 0)
                kl = bass.AP(s_ext.tensor, s_ext.offset,
                             [list(s_ext.ap[0]), [260, 4], [130, 2], [1, 2]])
                zin = bass.AP(zc.tensor, zc.offset,
                              [list(zc.ap[0]), [0, 4], [0, 2], [0, 2]])
                nc.vector.tensor_copy(kl, zin)
            else:
                kc = bass.AP(s_ext.tensor, s_ext.offset,
                             [list(s_ext.ap[0]), [260, 4], [130, 2]])
                zin = bass.AP(zc.tensor, zc.offset,
                              [list(zc.ap[0]), [0, 4], [0, 2]])
                nc.vector.tensor_copy(kc, zin)
                lc = bass.AP(s_ext.tensor, s_ext.offset + 1,
                             [list(s_ext.ap[0]), [260, 4], [130, 2]])
                cin = bass.AP(carry.tensor, carry.offset + 8 * p,
                              [list(carry.ap[0]), [2, 4], [1, 2]])
                nc.vector.tensor_copy(lc, cin)

            # A[r, (c01, blk)] for 129 blocks; one matmul per (ri, wi).
            for ri in range(4):
                aps = spool.tile([128, 258], F32, tag="aps")
                for wi in range(2):
                    lhsT = dm_sb[:, 512 * wi + 128 * ri:
                                 512 * wi + 128 * (ri + 1)]
                    v = xsq[:, CH * (2 * b0 + wi) + 2 * p:
                            CH * (2 * b0 + wi) + 2 * p + 2]
                    rhs = bass.AP(v.tensor, v.offset,
                                  [list(q) for q in v.ap] + [[2 * CH, 129]])
                    nc.tensor.matmul(aps[:], lhsT, rhs,
                                     start=(wi == 0), stop=(wi == 1))
                # PSUM exit (only one PSUM src allowed per vector op): copy A
                # to fp16 SBUF, split between Act and DVE to balance engines.
                a_sb = apool.tile([128, 258], F16, tag="a")
                if ri < act_copies:
                    nc.scalar.copy(a_sb[:], aps[:])
                else:
                    nc.vector.tensor_copy(a_sb[:], aps[:])
                # s[c01, f] = A[:, c01*129+f] +/- A[:, c01*129+f+1];
                # plain TensorTensor gets the DVE 2x fp16 mode.
                in0 = bass.AP(a_sb.tensor, a_sb.offset,
                              [list(a_sb.ap[0]), [129, 2], [1, 128]])
                in1 = bass.AP(a_sb.tensor, a_sb.offset + 1,
                              [list(a_sb.ap[0]), [129, 2], [1, 128]])
                outv = bass.AP(s_ext.tensor, s_ext.offset + 260 * ri + 2,
                               [list(s_ext.ap[0]), [130, 2], [1, 128]])
                op = (mybir.AluOpType.add if SIGMA[ri] > 0
                      else mybir.AluOpType.subtract)
                # the scan ISA only exists on DVE; Pool runs the late-segment
                # combines (off the critical path into the first scan chunk)
                tt_eng = nc.vector if ri < 2 else nc.gpsimd
                tt_eng.tensor_tensor(outv, in0, in1, op)

            # scan in two 520-col chunks so the iDFT's first accumulation
            # steps can start as soon as segments 0-1 are done
            yt = ypool.tile([128, 1040], F16, tag="y")
            for s2 in range(2):
                nc.vector.tensor_tensor_scan(
                    yt[:, 520 * s2:520 * (s2 + 1)],
                    s_ext[:, 520 * s2:520 * (s2 + 1)],
                    te_sb[:, 1040 * p + 520 * s2:1040 * p + 520 * (s2 + 1)],
                    0.0, mybir.AluOpType.add, mybir.AluOpType.mult)
            if half == 0:
                cs = bass.AP(carry.tensor, carry.offset + 8 * p,
                             [list(carry.ap[0]), [2, 4], [1, 2]])
                ys = bass.AP(yt.tensor, yt.offset + 129,
                             [list(yt.ap[0]), [260, 4], [130, 2]])
                nc.vector.tensor_copy(cs, ys)
            return yt

        def spectral_back(p, half, yt):
            # iDFT + hann + OLA; per-channel PSUM tiles so the tensor engine
            # is not gated on the other channel's tanh.
            oa = out_t.ap()
            for c01 in range(2):
                sl = opool.tile([128, 256], F32, tag="ops")
                for ri in range(4):
                    base = 260 * ri + 130 * c01
                    nc.tensor.matmul(sl[:], yt[:, base + 2:base + 130],
                                     em_sb[:, 512 * ri:512 * ri + 256],
                                     start=(ri == 0), stop=False)
                    nc.tensor.matmul(sl[:], yt[:, base + 1:base + 129],
                                     em_sb[:, 512 * ri + 256:512 * (ri + 1)],
                                     start=False, stop=(ri == 3))
                ot = outpool.tile([128, 256], F16, tag="ot")
                nc.scalar.activation(ot[:], sl[:],
                                     mybir.ActivationFunctionType.Tanh)
                nc.sync.dma_start(
                    oa[2 * p + c01, 128 * half:128 * (half + 1), :], ot[:])

        GSPLIT = 17          # mix groups needed before frame-half 0
        nc.sync.dma_start(m_sb[:], msb[:])
        for gt in range(2):
            load_group(gt)
        load_consts()
        load_text(0)
        for g in range(GSPLIT):
            if g % 2 == 0 and g // 2 + 2 < XG // 2:
                load_group(g // 2 + 2)
            mix_group(g)
        g_next = GSPLIT
        # half 0, software-pipelined by two pairs (the merged scan's latency
        # spans more than one tensor-engine stage); mix groups fill PE gaps
        done = []
        for p in range(NPAIR):
            if p % 2 == 0 and p // 2 + 1 < NPAIR // 2:
                load_text(p // 2 + 1)
            done.append(spectral_front(p, 0))
            if p >= 2:
                spectral_back(p - 2, 0, done[p - 2])
            if g_next < XG:
                if g_next % 2 == 0 and g_next // 2 + 2 < XG // 2:
                    load_group(g_next // 2 + 2)
                mix_group(g_next)
                g_next += 1
        while g_next < XG:
            mix_group(g_next)
            g_next += 1
        spectral_back(NPAIR - 2, 0, done[NPAIR - 2])
        spectral_back(NPAIR - 1, 0, done[NPAIR - 1])
        # half 1, software-pipelined by two pairs
        done = []
        for p in range(NPAIR):
            done.append(spectral_front(p, 1, act_copies=3))
            if p >= 2:
                spectral_back(p - 2, 1, done[p - 2])
        spectral_back(NPAIR - 2, 1, done[NPAIR - 2])
        spectral_back(NPAIR - 1, 1, done[NPAIR - 1])
    if split:
        _split_excess_waits(nc)
    return nc


_CACHE = {}


def _get_program():
    if "nc" not in _CACHE:
        _CACHE["nc"] = _build_program()
    return _CACHE["nc"]


def _host_inputs(x, mixer, transfer, gain):
    D, E1, E2 = _build_matrices()
    g = float(np.asarray(gain).reshape(-1)[0])
    dmat = np.ascontiguousarray(D, dtype=np.float16)
    emat = np.ascontiguousarray(
        np.concatenate([E1 * g, E2 * g], axis=1), dtype=np.float16)

    x = np.asarray(x, dtype=np.float32)
    mixer = np.asarray(mixer, dtype=np.float32)
    transfer = np.asarray(transfer, dtype=np.float32)

    rows = _packed_rows()
    r_of_row = np.array([r for (_, r) in rows], dtype=np.int64)

    in_maps = []
    for core in range(NCORES):
        b, h = divmod(core, 2)
        tr = transfer[CH * h:CH * (h + 1)]           # [32, 257]
        tt = tr[:, r_of_row].T.reshape(4, 128, CH)   # [ri, q, ch]
        text = np.zeros((128, NPAIR, 4, 260), dtype=np.float32)
        text[:, :, :, 1] = 1.0
        text[:, :, :, 131] = 1.0
        for p in range(NPAIR):
            for ri in range(4):
                text[:, p, ri, 2:130] = tt[ri, :, 2 * p][:, None]
                text[:, p, ri, 132:260] = tt[ri, :, 2 * p + 1][:, None]
        in_maps.append({
            "xin": np.ascontiguousarray(x[b], dtype=np.float16),
            "msb": np.ascontiguousarray(mixer[:, CH * h:CH * (h + 1)],
                                        dtype=np.float16),
            "dmat": dmat,
            "emat": emat,
            "text": np.ascontiguousarray(
                text.reshape(128, NPAIR * 1040), dtype=np.float16),
        })
    return in_maps


def _run(in_maps, trace=False):
    nc = _get_program()
    return run_bass_kernel_spmd(nc, in_maps, list(range(NCORES)), trace=trace)


def kernel(x, mixer, transfer, gain, _trace=False):
    in_maps = _host_inputs(x, mixer, transfer, gain)
    res = _run(in_maps, trace=_trace)
    out = np.empty((B, C, T), dtype=np.float32)
    for core in range(NCORES):
        b, h = divmod(core, 2)
        out[b, CH * h:CH * (h + 1), :] = \
            res.results[core]["out"].astype(np.float32).reshape(CH, T)
    if _trace:
        _CACHE["last_result"] = res
    return out


# revision 25
# speedup vs baseline: 1.0288x; 1.0288x over previous
"""Trainium2 Bass kernel for nn_Block_68633577390412.

Pipeline (reference): channel mix (64x64) -> frame into 256 half-overlapping
512-windows -> rfft -> per-(c,k) IIR over frames y_f = (s_f + y_{f-1})*t ->
irfft -> hann window -> overlap-add -> tanh(gain*.)

Sharding: 8 cores = 4 batches x 2 channel-halves, no cross-core traffic.

Per-core design (v2 — hop-block DFT + fp16 datapath):
  - Adjacent frames overlap by half a window (hop 256), so the rfft of frame
    f decomposes into two half-window transforms: S_f = A_f + (-1)^r A_{f+1}
    where A_h[r] = sum_{w<256} x[256h+w] e^{-i 2pi r w/512}. Each hop block's
    A is computed once (K=256 contraction, half the tensor-engine streams of
    the direct K=512 frame DFT); the +/- combine is a single fused
    scalar_tensor_tensor sweep on the vector engine.
  - Packed-real spectral rows are grouped by frequency parity
    (C0=Re-even, C1=Re-odd, C2=[Re_256|Im-even], C3=Im-odd) so (-1)^r is a
    constant sign per 128-row chunk and the combine needs no row masks.
  - The whole spectral datapath runs in fp16 (inputs, DFT bases, scan state
    s/y, transfer t, output): fp16 keeps ~1e-3 relative accuracy (tolerance
    is 2e-2) and unlocks the DVE 2x/4x wide modes plus half-size DMA.
  - The frame recurrence is the DVE TensorTensorScan instruction with fp32
    internal state, reading fp16 s tiles from SBUF.
  - irfft + hann + overlap-add stay fused as two accumulating matmul streams
    per output tile against precomputed E1/E2 = (irfft-basis * hann * gain)
    halves; tanh runs on the scalar engine over both channels at once and
    the output DMA stores fp16 with a (frame, channel, w) interleaved view.
  - All DMA goes through HWDGE (SP-engine issue, ~0.6us shared-device
    overhead) instead of Pool SWDGE (~1us Pool-engine hold each), freeing
    the Pool engine to zero scan-boundary columns.
  - Mix runs as a single fp16 stream (K=64) into PSUM, copied to the
    time-on-partitions fp16 signal buffer by the scalar engine; spectral
    work starts once the first ~17/32 of the signal is mixed and the
    remaining mix groups fill tensor-engine gaps between spectral pairs.
  - TRN2 instructions carry a single sync-wait slot; a post-pass hoists
    excess Tile-assigned waits onto standalone event-semaphore waits.
"""
import numpy as np
import ml_dtypes
from contextlib import ExitStack

import concourse.bass as bass
import concourse.tile as tile
from concourse import mybir
from concourse.bass_utils import run_bass_kernel_spmd

F32 = mybir.dt.float32
F16 = mybir.dt.float16

B, C, T = 4, 64, 65536
WS, STEP, K, NF = 512, 256, 257, 256
NCORES = 8
CH = 32            # channels per core
NCHUNK = 512       # 128-sample time chunks in T
XG = 32            # mix groups (2048 samples each)
NPAIR = CH // 2    # channel pairs per core

SIGMA = [1.0, -1.0, 1.0, -1.0]   # (-1)^r per packed row chunk C0..C3


def _packed_rows():
    """Packed-real row order: (type, r) per packed row, grouped so that
    (-1)^r is constant per 128-row chunk."""
    rows = []
    rows += [("re", r) for r in range(0, 256, 2)]          # C0: Re even
    rows += [("re", r) for r in range(1, 256, 2)]          # C1: Re odd
    rows += [("re", 256)] + [("im", r) for r in range(2, 256, 2)]  # C2
    rows += [("im", r) for r in range(1, 256, 2)]          # C3: Im odd
    assert len(rows) == 512
    return rows


def _build_matrices():
    rows = _packed_rows()
    w = np.arange(WS, dtype=np.float64)
    wh = w[:STEP]
    # D~ [256, 512]: half-window DFT basis, packed/parity-ordered columns.
    D = np.empty((STEP, 512), dtype=np.float64)
    for j, (ty, r) in enumerate(rows):
        if ty == "re":
            D[:, j] = np.cos(2 * np.pi * r * wh / WS)
        else:
            D[:, j] = -np.sin(2 * np.pi * r * wh / WS)
    # G [512, 512]: packed irfft basis rows in the same order.
    G = np.empty((512, WS), dtype=np.float64)
    for j, (ty, r) in enumerate(rows):
        if ty == "re":
            coef = 1.0 / WS if r in (0, 256) else 2.0 / WS
            G[j, :] = coef * np.cos(2 * np.pi * r * w / WS)
        else:
            G[j, :] = -(2.0 / WS) * np.sin(2 * np.pi * r * w / WS)
    hann = 0.5 * (1.0 - np.cos(2.0 * np.pi * w / WS))
    E1 = G[:, :STEP] * hann[None, :STEP]
    E2 = G[:, STEP:] * hann[None, STEP:]
    return D, E1, E2


def _split_excess_waits(nc):
    """TRN2 instructions have a single sync-wait slot. Tile's semaphore
    assignment can attach several waits to one instruction; hoist the extras
    onto standalone event-semaphore wait instructions on the same engine."""
    ctr = 0
    for fn in nc.m.functions:
        for bb in fn.blocks:
            out = []
            changed = False
            for inst in bb.instructions:
                si = inst.sync_info
                ow = list(si.on_wait) if si and si.on_wait else []
                if len(ow) > 1:
                    for wv in ow[:-1]:
                        ev = mybir.InstEventSemaphore(
                            name=f"WSPLIT-{ctr}", ins=[], outs=[])
                        ctr += 1
                        ev.engine = inst.engine
                        evsi = ev.sync_info
                        if evsi is None:
                            ev.sync_info = mybir.SyncInfo(on_wait=[wv],
                                                          on_update=[])
                        else:
                            evsi.on_wait = [wv]
                        out.append(ev)
                    si.on_wait = [ow[-1]]
                    changed = True
                out.append(inst)
            if changed:
                bb.instructions = out
    return ctr


def _build_program(split=True):
    nc = bass.Bass("TRN2", target_bir_lowering=False, debug=False,
                   num_devices=NCORES)
    xin_t = nc.dram_tensor("xin", [C, T], F16, kind="ExternalInput")
    msb_t = nc.dram_tensor("msb", [C, CH], F16, kind="ExternalInput")
    dmat_t = nc.dram_tensor("dmat", [STEP, 512], F16, kind="ExternalInput")
    emat_t = nc.dram_tensor("emat", [512, 512], F16, kind="ExternalInput")
    # t_ext: per channel pair, 4 segments [0, 1, t(c0)*128, 0, 1, t(c1)*128]
    # (the 0/1 columns reset the merged scan's state / load the carry)
    text_t = nc.dram_tensor("text", [128, NPAIR * 1040], F16,
                            kind="ExternalInput")
    out_t = nc.dram_tensor("out", [CH, 256, 256], F16, kind="ExternalOutput")

    xin = xin_t.ap()
    msb, dmat, emat, text = msb_t.ap(), dmat_t.ap(), emat_t.ap(), text_t.ap()

    with tile.TileContext(nc) as tc, ExitStack() as ctx:
        cpool = ctx.enter_context(tc.tile_pool(name="const", bufs=1))
        m_sb = cpool.tile([C, CH], F16, tag="m_sb")
        dm_sb = cpool.tile([128, 1024], F16, tag="dm_sb")
        em_sb = cpool.tile([128, 2048], F16, tag="em_sb")
        te_sb = cpool.tile([128, NPAIR * 1040], F16, tag="te_sb")
        zc = cpool.tile([128, 1], F16, tag="zc")
        nc.vector.memset(zc[:], 0.0)

        # x'' : mixed signal, time-on-partitions. col = 32*chunk + d_local.
        xsq = cpool.tile([128, CH * (NCHUNK + 2)], F16, tag="xsq")
        nc.vector.memset(xsq[:, CH * NCHUNK:], 0.0)

        xpool = ctx.enter_context(tc.tile_pool(name="xin", bufs=4))

        def load_consts():
            # issued after the first x tiles: nothing here is needed until
            # the first DFT ~12us in, and HWDGE issue order gates the input.
            for i in range(2):
                nc.sync.dma_start(dm_sb[:, 512 * i:512 * (i + 1)],
                                  dmat[128 * i:128 * (i + 1), :])
            for i in range(4):
                nc.sync.dma_start(em_sb[:, 512 * i:512 * (i + 1)],
                                  emat[128 * i:128 * (i + 1), :])

        def load_text(j):
            # one slice covers channel pairs 2j, 2j+1
            nc.sync.dma_start(te_sb[:, 2080 * j:2080 * (j + 1)],
                              text[:, 2080 * j:2080 * (j + 1)])
        mixpool = ctx.enter_context(tc.tile_pool(name="mixps", bufs=2,
                                                 space="PSUM"))
        spool = ctx.enter_context(tc.tile_pool(name="aps", bufs=4,
                                               space="PSUM"))
        opool = ctx.enter_context(tc.tile_pool(name="ops", bufs=2,
                                               space="PSUM"))
        apool = ctx.enter_context(tc.tile_pool(name="asb", bufs=8))
        sbpool = ctx.enter_context(tc.tile_pool(name="ssb", bufs=4))
        ypool = ctx.enter_context(tc.tile_pool(name="y", bufs=4))
        outpool = ctx.enter_context(tc.tile_pool(name="outs", bufs=4))
        # scan carry between frame halves; col = 8p + 2*ri + c01
        carry = cpool.tile([128, 8 * NPAIR], F16, tag="carry")

        xtiles = [None] * (XG // 2)

        def load_group(gt):
            xt = xpool.tile([C, 4096], F16, tag="xt")
            nc.sync.dma_start(xt[:], xin[:, 4096 * gt:4096 * (gt + 1)])
            xtiles[gt] = xt

        def mix_group(g):
            xt = xtiles[g // 2]
            base = 2048 * (g % 2)
            mps = mixpool.tile([128, 512], F32, tag="mps")
            for m in range(16):
                lhsT = xt[:, base + 128 * m:base + 128 * (m + 1)]
                nc.tensor.matmul(mps[:, CH * m:CH * (m + 1)], lhsT, m_sb[:],
                                 start=True, stop=True)
            nc.scalar.copy(xsq[:, 512 * g:512 * (g + 1)], mps[:])

        def spectral_front(p, half, act_copies=2):
            b0 = 128 * half            # first hop block of this half
            # s_ext: 4 segments of 260 cols: [K, L, s(c0)*128, K, L, s(c1)*128]
            # K has t=0 (kills the merged scan's state across segments), L has
            # t=1 with s = the carry value, so one scan instruction covers all
            # 8 (c01, ri) recurrences and the L columns double as the y_{-1}
            # boundary column that the E2 overlap-add stream reads.
            s_ext = sbpool.tile([128, 1040], F16, tag="s")
            if half == 0:
                # zero all K and L columns (carry-in is 0)
                kl = bass.AP(s_ext.tensor, s_ext.offset,
                             [list(s_ext.ap[0]), [260, 4], [130, 2], [1, 2]])
                zin = bass.AP(zc.tensor, zc.offset,
                              [list(zc.ap[0]), [0, 4], [0, 2], [0, 2]])
                nc.vector.tensor_copy(kl, zin)
            else:
                kc = bass.AP(s_ext.tensor, s_ext.offset,
                             [list(s_ext.ap[0]), [260, 4], [130, 2]])
                zin = bass.AP(zc.tensor, zc.offset,
                              [list(zc.ap[0]), [0, 4], [0, 2]])
                nc.vector.tensor_copy(kc, zin)
                lc = bass.AP(s_ext.tensor, s_ext.offset + 1,
                             [list(s_ext.ap[0]), [260, 4], [130, 2]])
                cin = bass.AP(carry.tensor, carry.offset + 8 * p,
                              [list(carry.ap[0]), [2, 4], [1, 2]])
                nc.vector.tensor_copy(lc, cin)

            # A[r, (c01, blk)] for 129 blocks; one matmul per (ri, wi).
            for ri in range(4):
                aps = spool.tile([128, 258], F32, tag="aps")
                for wi in range(2):
                    lhsT = dm_sb[:, 512 * wi + 128 * ri:
                                 512 * wi + 128 * (ri + 1)]
                    v = xsq[:, CH * (2 * b0 + wi) + 2 * p:
                            CH * (2 * b0 + wi) + 2 * p + 2]
                    rhs = bass.AP(v.tensor, v.offset,
                                  [list(q) for q in v.ap] + [[2 * CH, 129]])
                    nc.tensor.matmul(aps[:], lhsT, rhs,
                                     start=(wi == 0), stop=(wi == 1))
                # PSUM exit (only one PSUM src allowed per vector op): copy A
                # to fp16 SBUF, split between Act and DVE to balance engines.
                a_sb = apool.tile([128, 258], F16, tag="a")
                if ri < act_copies:
                    nc.scalar.copy(a_sb[:], aps[:])
                else:
                    nc.vector.tensor_copy(a_sb[:], aps[:])
                # s[c01, f] = A[:, c01*129+f] +/- A[:, c01*129+f+1];
                # plain TensorTensor gets the DVE 2x fp16 mode.
                in0 = bass.AP(a_sb.tensor, a_sb.offset,
                              [list(a_sb.ap[0]), [129, 2], [1, 128]])
                in1 = bass.AP(a_sb.tensor, a_sb.offset + 1,
                              [list(a_sb.ap[0]), [129, 2], [1, 128]])
                outv = bass.AP(s_ext.tensor, s_ext.offset + 260 * ri + 2,
                               [list(s_ext.ap[0]), [130, 2], [1, 128]])
                op = (mybir.AluOpType.add if SIGMA[ri] > 0
                      else mybir.AluOpType.subtract)
                # the scan ISA only exists on DVE; Pool runs the late-segment
                # combines (off the critical path into the first scan chunk)
                tt_eng = nc.vector if ri < 2 else nc.gpsimd
                tt_eng.tensor_tensor(outv, in0, in1, op)

            # scan per 260-col segment: each starts as soon as its combine is
            # done and feeds its iDFT accumulation step without waiting for
            # the other segments
            yt = ypool.tile([128, 1040], F16, tag="y")
            for ri in range(4):
                nc.vector.tensor_tensor_scan(
                    yt[:, 260 * ri:260 * (ri + 1)],
                    s_ext[:, 260 * ri:260 * (ri + 1)],
                    te_sb[:, 1040 * p + 260 * ri:1040 * p + 260 * (ri + 1)],
                    0.0, mybir.AluOpType.add, mybir.AluOpType.mult)
            if half == 0:
                cs = bass.AP(carry.tensor, carry.offset + 8 * p,
                             [list(carry.ap[0]), [2, 4], [1, 2]])
                ys = bass.AP(yt.tensor, yt.offset + 129,
                             [list(yt.ap[0]), [260, 4], [130, 2]])
                nc.vector.tensor_copy(cs, ys)
            return yt

        def spectral_back(p, half, yt):
            # iDFT + hann + OLA; per-channel PSUM tiles so the tensor engine
            # is not gated on the other channel's tanh.
            oa = out_t.ap()
            for c01 in range(2):
                sl = opool.tile([128, 256], F32, tag="ops")
                for ri in range(4):
                    base = 260 * ri + 130 * c01
                    nc.tensor.matmul(sl[:], yt[:, base + 2:base + 130],
                                     em_sb[:, 512 * ri:512 * ri + 256],
                                     start=(ri == 0), stop=False)
                    nc.tensor.matmul(sl[:], yt[:, base + 1:base + 129],
                                     em_sb[:, 512 * ri + 256:512 * (ri + 1)],
                                     start=False, stop=(ri == 3))
                ot = outpool.tile([128, 256], F16, tag="ot")
                nc.scalar.activation(ot[:], sl[:],
                                     mybir.ActivationFunctionType.Tanh)
                nc.sync.dma_start(
                    oa[2 * p + c01, 128 * half:128 * (half + 1), :], ot[:])

        GSPLIT = 17          # mix groups needed before frame-half 0
        nc.sync.dma_start(m_sb[:], msb[:])
        for gt in range(2):
            load_group(gt)
        load_consts()
        load_text(0)
        for g in range(GSPLIT):
            if g % 2 == 0 and g // 2 + 2 < XG // 2:
                load_group(g // 2 + 2)
            mix_group(g)
        g_next = GSPLIT
        # half 0, software-pipelined by two pairs (the merged scan's latency
        # spans more than one tensor-engine stage); mix groups fill PE gaps
        done = []
        for p in range(NPAIR):
            if p % 2 == 0 and p // 2 + 1 < NPAIR // 2:
                load_text(p // 2 + 1)
            done.append(spectral_front(p, 0))
            if p >= 2:
                spectral_back(p - 2, 0, done[p - 2])
            if g_next < XG:
                if g_next % 2 == 0 and g_next // 2 + 2 < XG // 2:
                    load_group(g_next // 2 + 2)
                mix_group(g_next)
                g_next += 1
        while g_next < XG:
            mix_group(g_next)
            g_next += 1
        spectral_back(NPAIR - 2, 0, done[NPAIR - 2])
        spectral_back(NPAIR - 1, 0, done[NPAIR - 1])
        # half 1, software-pipelined by two pairs
        done = []
        for p in range(NPAIR):
            done.append(spectral_front(p, 1, act_copies=3))
            if p >= 2:
                spectral_back(p - 2, 1, done[p - 2])
        spectral_back(NPAIR - 2, 1, done[NPAIR - 2])
        spectral_back(NPAIR - 1, 1, done[NPAIR - 1])
    if split:
        _split_excess_waits(nc)
    return nc


_CACHE = {}


def _get_program():
    if "nc" not in _CACHE:
        _CACHE["nc"] = _build_program()
    return _CACHE["nc"]


def _host_inputs(x, mixer, transfer, gain):
    D, E1, E2 = _build_matrices()
    g = float(np.asarray(gain).reshape(-1)[0])
    dmat = np.ascontiguousarray(D, dtype=np.float16)
    emat = np.ascontiguousarray(
        np.concatenate([E1 * g, E2 * g], axis=1), dtype=np.float16)

    x = np.asarray(x, dtype=np.float32)
    mixer = np.asarray(mixer, dtype=np.float32)
    transfer = np.asarray(transfer, dtype=np.float32)

    rows = _packed_rows()
    r_of_row = np.array([r for (_, r) in rows], dtype=np.int64)

    in_maps = []
    for core in range(NCORES):
        b, h = divmod(core, 2)
        tr = transfer[CH * h:CH * (h + 1)]           # [32, 257]
        tt = tr[:, r_of_row].T.reshape(4, 128, CH)   # [ri, q, ch]
        text = np.zeros((128, NPAIR, 4, 260), dtype=np.float32)
        text[:, :, :, 1] = 1.0
        text[:, :, :, 131] = 1.0
        for p in range(NPAIR):
            for ri in range(4):
                text[:, p, ri, 2:130] = tt[ri, :, 2 * p][:, None]
                text[:, p, ri, 132:260] = tt[ri, :, 2 * p + 1][:, None]
        in_maps.append({
            "xin": np.ascontiguousarray(x[b], dtype=np.float16),
            "msb": np.ascontiguousarray(mixer[:, CH * h:CH * (h + 1)],
                                        dtype=np.float16),
            "dmat": dmat,
            "emat": emat,
            "text": np.ascontiguousarray(
                text.reshape(128, NPAIR * 1040), dtype=np.float16),
        })
    return in_maps


def _run(in_maps, trace=False):
    nc = _get_program()
    return run_bass_kernel_spmd(nc, in_maps, list(range(NCORES)), trace=trace)


def kernel(x, mixer, transfer, gain, _trace=False):
    in_maps = _host_inputs(x, mixer, transfer, gain)
    res = _run(in_maps, trace=_trace)
    out = np.empty((B, C, T), dtype=np.float32)
    for core in range(NCORES):
        b, h = divmod(core, 2)
        out[b, CH * h:CH * (h + 1), :] = \
            res.results[core]["out"].astype(np.float32).reshape(CH, T)
    if _trace:
        _CACHE["last_result"] = res
    return out


# revision 26
# speedup vs baseline: 1.0301x; 1.0012x over previous
"""Trainium2 Bass kernel for nn_Block_68633577390412.

Pipeline (reference): channel mix (64x64) -> frame into 256 half-overlapping
512-windows -> rfft -> per-(c,k) IIR over frames y_f = (s_f + y_{f-1})*t ->
irfft -> hann window -> overlap-add -> tanh(gain*.)

Sharding: 8 cores = 4 batches x 2 channel-halves, no cross-core traffic.

Per-core design (v2 — hop-block DFT + fp16 datapath):
  - Adjacent frames overlap by half a window (hop 256), so the rfft of frame
    f decomposes into two half-window transforms: S_f = A_f + (-1)^r A_{f+1}
    where A_h[r] = sum_{w<256} x[256h+w] e^{-i 2pi r w/512}. Each hop block's
    A is computed once (K=256 contraction, half the tensor-engine streams of
    the direct K=512 frame DFT); the +/- combine is a single fused
    scalar_tensor_tensor sweep on the vector engine.
  - Packed-real spectral rows are grouped by frequency parity
    (C0=Re-even, C1=Re-odd, C2=[Re_256|Im-even], C3=Im-odd) so (-1)^r is a
    constant sign per 128-row chunk and the combine needs no row masks.
  - The whole spectral datapath runs in fp16 (inputs, DFT bases, scan state
    s/y, transfer t, output): fp16 keeps ~1e-3 relative accuracy (tolerance
    is 2e-2) and unlocks the DVE 2x/4x wide modes plus half-size DMA.
  - The frame recurrence is the DVE TensorTensorScan instruction with fp32
    internal state, reading fp16 s tiles from SBUF.
  - irfft + hann + overlap-add stay fused as two accumulating matmul streams
    per output tile against precomputed E1/E2 = (irfft-basis * hann * gain)
    halves; tanh runs on the scalar engine over both channels at once and
    the output DMA stores fp16 with a (frame, channel, w) interleaved view.
  - All DMA goes through HWDGE (SP-engine issue, ~0.6us shared-device
    overhead) instead of Pool SWDGE (~1us Pool-engine hold each), freeing
    the Pool engine to zero scan-boundary columns.
  - Mix runs as a single fp16 stream (K=64) into PSUM, copied to the
    time-on-partitions fp16 signal buffer by the scalar engine; spectral
    work starts once the first ~17/32 of the signal is mixed and the
    remaining mix groups fill tensor-engine gaps between spectral pairs.
  - TRN2 instructions carry a single sync-wait slot; a post-pass hoists
    excess Tile-assigned waits onto standalone event-semaphore waits.
"""
import numpy as np
import ml_dtypes
from contextlib import ExitStack

import concourse.bass as bass
import concourse.tile as tile
from concourse import mybir
from concourse.bass_utils import run_bass_kernel_spmd

F32 = mybir.dt.float32
F16 = mybir.dt.float16

B, C, T = 4, 64, 65536
WS, STEP, K, NF = 512, 256, 257, 256
NCORES = 8
CH = 32            # channels per core
NCHUNK = 512       # 128-sample time chunks in T
XG = 32            # mix groups (2048 samples each)
NPAIR = CH // 2    # channel pairs per core

SIGMA = [1.0, -1.0, 1.0, -1.0]   # (-1)^r per packed row chunk C0..C3


def _packed_rows():
    """Packed-real row order: (type, r) per packed row, grouped so that
    (-1)^r is constant per 128-row chunk."""
    rows = []
    rows += [("re", r) for r in range(0, 256, 2)]          # C0: Re even
    rows += [("re", r) for r in range(1, 256, 2)]          # C1: Re odd
    rows += [("re", 256)] + [("im", r) for r in range(2, 256, 2)]  # C2
    rows += [("im", r) for r in range(1, 256, 2)]          # C3: Im odd
    assert len(rows) == 512
    return rows


def _build_matrices():
    rows = _packed_rows()
    w = np.arange(WS, dtype=np.float64)
    wh = w[:STEP]
    # D~ [256, 512]: half-window DFT basis, packed/parity-ordered columns.
    D = np.empty((STEP, 512), dtype=np.float64)
    for j, (ty, r) in enumerate(rows):
        if ty == "re":
            D[:, j] = np.cos(2 * np.pi * r * wh / WS)
        else:
            D[:, j] = -np.sin(2 * np.pi * r * wh / WS)
    # G [512, 512]: packed irfft basis rows in the same order.
    G = np.empty((512, WS), dtype=np.float64)
    for j, (ty, r) in enumerate(rows):
        if ty == "re":
            coef = 1.0 / WS if r in (0, 256) else 2.0 / WS
            G[j, :] = coef * np.cos(2 * np.pi * r * w / WS)
        else:
            G[j, :] = -(2.0 / WS) * np.sin(2 * np.pi * r * w / WS)
    hann = 0.5 * (1.0 - np.cos(2.0 * np.pi * w / WS))
    E1 = G[:, :STEP] * hann[None, :STEP]
    E2 = G[:, STEP:] * hann[None, STEP:]
    return D, E1, E2


def _split_excess_waits(nc):
    """TRN2 instructions have a single sync-wait slot. Tile's semaphore
    assignment can attach several waits to one instruction; hoist the extras
    onto standalone event-semaphore wait instructions on the same engine."""
    ctr = 0
    for fn in nc.m.functions:
        for bb in fn.blocks:
            out = []
            changed = False
            for inst in bb.instructions:
                si = inst.sync_info
                ow = list(si.on_wait) if si and si.on_wait else []
                if len(ow) > 1:
                    for wv in ow[:-1]:
                        ev = mybir.InstEventSemaphore(
                            name=f"WSPLIT-{ctr}", ins=[], outs=[])
                        ctr += 1
                        ev.engine = inst.engine
                        evsi = ev.sync_info
                        if evsi is None:
                            ev.sync_info = mybir.SyncInfo(on_wait=[wv],
                                                          on_update=[])
                        else:
                            evsi.on_wait = [wv]
                        out.append(ev)
                    si.on_wait = [ow[-1]]
                    changed = True
                out.append(inst)
            if changed:
                bb.instructions = out
    return ctr


def _build_program(split=True):
    nc = bass.Bass("TRN2", target_bir_lowering=False, debug=False,
                   num_devices=NCORES)
    xin_t = nc.dram_tensor("xin", [C, T], F16, kind="ExternalInput")
    msb_t = nc.dram_tensor("msb", [C, CH], F16, kind="ExternalInput")
    dmat_t = nc.dram_tensor("dmat", [STEP, 512], F16, kind="ExternalInput")
    emat_t = nc.dram_tensor("emat", [512, 512], F16, kind="ExternalInput")
    # t_ext: per channel pair, 4 segments [0, 1, t(c0)*128, 0, 1, t(c1)*128]
    # (the 0/1 columns reset the merged scan's state / load the carry)
    text_t = nc.dram_tensor("text", [128, NPAIR * 1040], F16,
                            kind="ExternalInput")
    out_t = nc.dram_tensor("out", [CH, 256, 256], F16, kind="ExternalOutput")

    xin = xin_t.ap()
    msb, dmat, emat, text = msb_t.ap(), dmat_t.ap(), emat_t.ap(), text_t.ap()

    with tile.TileContext(nc) as tc, ExitStack() as ctx:
        cpool = ctx.enter_context(tc.tile_pool(name="const", bufs=1))
        m_sb = cpool.tile([C, CH], F16, tag="m_sb")
        dm_sb = cpool.tile([128, 1024], F16, tag="dm_sb")
        em_sb = cpool.tile([128, 2048], F16, tag="em_sb")
        te_sb = cpool.tile([128, NPAIR * 1040], F16, tag="te_sb")
        zc = cpool.tile([128, 1], F16, tag="zc")
        nc.vector.memset(zc[:], 0.0)

        # x'' : mixed signal, time-on-partitions. col = 32*chunk + d_local.
        xsq = cpool.tile([128, CH * (NCHUNK + 2)], F16, tag="xsq")
        nc.vector.memset(xsq[:, CH * NCHUNK:], 0.0)

        xpool = ctx.enter_context(tc.tile_pool(name="xin", bufs=4))

        def load_consts():
            # issued after the first x tiles: nothing here is needed until
            # the first DFT ~12us in, and HWDGE issue order gates the input.
            for i in range(2):
                nc.sync.dma_start(dm_sb[:, 512 * i:512 * (i + 1)],
                                  dmat[128 * i:128 * (i + 1), :])
            for i in range(4):
                nc.sync.dma_start(em_sb[:, 512 * i:512 * (i + 1)],
                                  emat[128 * i:128 * (i + 1), :])

        def load_text(j):
            # one slice covers channel pairs 2j, 2j+1
            nc.sync.dma_start(te_sb[:, 2080 * j:2080 * (j + 1)],
                              text[:, 2080 * j:2080 * (j + 1)])
        mixpool = ctx.enter_context(tc.tile_pool(name="mixps", bufs=2,
                                                 space="PSUM"))
        spool = ctx.enter_context(tc.tile_pool(name="aps", bufs=4,
                                               space="PSUM"))
        opool = ctx.enter_context(tc.tile_pool(name="ops", bufs=2,
                                               space="PSUM"))
        apool = ctx.enter_context(tc.tile_pool(name="asb", bufs=8))
        sbpool = ctx.enter_context(tc.tile_pool(name="ssb", bufs=4))
        ypool = ctx.enter_context(tc.tile_pool(name="y", bufs=4))
        outpool = ctx.enter_context(tc.tile_pool(name="outs", bufs=4))
        # scan carry between frame halves; col = 8p + 2*ri + c01
        carry = cpool.tile([128, 8 * NPAIR], F16, tag="carry")

        xtiles = [None] * (XG // 2)

        def load_group(gt):
            xt = xpool.tile([C, 4096], F16, tag="xt")
            nc.sync.dma_start(xt[:], xin[:, 4096 * gt:4096 * (gt + 1)])
            xtiles[gt] = xt

        def mix_group(g):
            xt = xtiles[g // 2]
            base = 2048 * (g % 2)
            mps = mixpool.tile([128, 512], F32, tag="mps")
            for m in range(16):
                lhsT = xt[:, base + 128 * m:base + 128 * (m + 1)]
                nc.tensor.matmul(mps[:, CH * m:CH * (m + 1)], lhsT, m_sb[:],
                                 start=True, stop=True)
            nc.scalar.copy(xsq[:, 512 * g:512 * (g + 1)], mps[:])

        def spectral_front(p, half, act_copies=2):
            b0 = 128 * half            # first hop block of this half
            # s_ext: 4 segments of 260 cols: [K, L, s(c0)*128, K, L, s(c1)*128]
            # K has t=0 (kills the merged scan's state across segments), L has
            # t=1 with s = the carry value, so one scan instruction covers all
            # 8 (c01, ri) recurrences and the L columns double as the y_{-1}
            # boundary column that the E2 overlap-add stream reads.
            s_ext = sbpool.tile([128, 1040], F16, tag="s")
            if half == 0:
                # zero all K and L columns (carry-in is 0)
                kl = bass.AP(s_ext.tensor, s_ext.offset,
                             [list(s_ext.ap[0]), [260, 4], [130, 2], [1, 2]])
                zin = bass.AP(zc.tensor, zc.offset,
                              [list(zc.ap[0]), [0, 4], [0, 2], [0, 2]])
                nc.vector.tensor_copy(kl, zin)
            else:
                kc = bass.AP(s_ext.tensor, s_ext.offset,
                             [list(s_ext.ap[0]), [260, 4], [130, 2]])
                zin = bass.AP(zc.tensor, zc.offset,
                              [list(zc.ap[0]), [0, 4], [0, 2]])
                nc.vector.tensor_copy(kc, zin)
                lc = bass.AP(s_ext.tensor, s_ext.offset + 1,
                             [list(s_ext.ap[0]), [260, 4], [130, 2]])
                cin = bass.AP(carry.tensor, carry.offset + 8 * p,
                              [list(carry.ap[0]), [2, 4], [1, 2]])
                nc.vector.tensor_copy(lc, cin)

            # A[r, (c01, blk)] for 129 blocks; one matmul per (ri, wi).
            for ri in range(4):
                aps = spool.tile([128, 258], F32, tag="aps")
                for wi in range(2):
                    lhsT = dm_sb[:, 512 * wi + 128 * ri:
                                 512 * wi + 128 * (ri + 1)]
                    v = xsq[:, CH * (2 * b0 + wi) + 2 * p:
                            CH * (2 * b0 + wi) + 2 * p + 2]
                    rhs = bass.AP(v.tensor, v.offset,
                                  [list(q) for q in v.ap] + [[2 * CH, 129]])
                    nc.tensor.matmul(aps[:], lhsT, rhs,
                                     start=(wi == 0), stop=(wi == 1))
                # PSUM exit (only one PSUM src allowed per vector op): copy A
                # to fp16 SBUF, split between Act and DVE to balance engines.
                a_sb = apool.tile([128, 258], F16, tag="a")
                if ri < act_copies:
                    nc.scalar.copy(a_sb[:], aps[:])
                else:
                    nc.vector.tensor_copy(a_sb[:], aps[:])
                # s[c01, f] = A[:, c01*129+f] +/- A[:, c01*129+f+1];
                # plain TensorTensor gets the DVE 2x fp16 mode.
                in0 = bass.AP(a_sb.tensor, a_sb.offset,
                              [list(a_sb.ap[0]), [129, 2], [1, 128]])
                in1 = bass.AP(a_sb.tensor, a_sb.offset + 1,
                              [list(a_sb.ap[0]), [129, 2], [1, 128]])
                outv = bass.AP(s_ext.tensor, s_ext.offset + 260 * ri + 2,
                               [list(s_ext.ap[0]), [130, 2], [1, 128]])
                op = (mybir.AluOpType.add if SIGMA[ri] > 0
                      else mybir.AluOpType.subtract)
                # the scan ISA only exists on DVE; Pool runs the late-segment
                # combines (off the critical path into the first scan chunk)
                tt_eng = nc.vector if ri < 2 else nc.gpsimd
                tt_eng.tensor_tensor(outv, in0, in1, op)

            # scan per 260-col segment: each starts as soon as its combine is
            # done and feeds its iDFT accumulation step without waiting for
            # the other segments
            yt = ypool.tile([128, 1040], F16, tag="y")
            for ri in range(4):
                nc.vector.tensor_tensor_scan(
                    yt[:, 260 * ri:260 * (ri + 1)],
                    s_ext[:, 260 * ri:260 * (ri + 1)],
                    te_sb[:, 1040 * p + 260 * ri:1040 * p + 260 * (ri + 1)],
                    0.0, mybir.AluOpType.add, mybir.AluOpType.mult)
            if half == 0:
                cs = bass.AP(carry.tensor, carry.offset + 8 * p,
                             [list(carry.ap[0]), [2, 4], [1, 2]])
                ys = bass.AP(yt.tensor, yt.offset + 129,
                             [list(yt.ap[0]), [260, 4], [130, 2]])
                nc.vector.tensor_copy(cs, ys)
            return yt

        def spectral_back(p, half, yt):
            # iDFT + hann + OLA; per-channel PSUM tiles so the tensor engine
            # is not gated on the other channel's tanh.
            oa = out_t.ap()
            for c01 in range(2):
                sl = opool.tile([128, 256], F32, tag="ops")
                for ri in range(4):
                    base = 260 * ri + 130 * c01
                    nc.tensor.matmul(sl[:], yt[:, base + 2:base + 130],
                                     em_sb[:, 512 * ri:512 * ri + 256],
                                     start=(ri == 0), stop=False)
                    nc.tensor.matmul(sl[:], yt[:, base + 1:base + 129],
                                     em_sb[:, 512 * ri + 256:512 * (ri + 1)],
                                     start=False, stop=(ri == 3))
                ot = outpool.tile([128, 256], F16, tag="ot")
                nc.scalar.activation(ot[:], sl[:],
                                     mybir.ActivationFunctionType.Tanh)
                nc.sync.dma_start(
                    oa[2 * p + c01, 128 * half:128 * (half + 1), :], ot[:])

        GSPLIT = 17          # mix groups needed before frame-half 0
        nc.sync.dma_start(m_sb[:], msb[:])
        for gt in range(2):
            load_group(gt)
        load_consts()
        load_text(0)
        for g in range(GSPLIT):
            if g % 2 == 0 and g // 2 + 2 < XG // 2:
                load_group(g // 2 + 2)
            mix_group(g)
        g_next = GSPLIT
        # half 0, software-pipelined by two pairs (the merged scan's latency
        # spans more than one tensor-engine stage); mix groups fill PE gaps
        done = []
        for p in range(NPAIR):
            if p % 2 == 0 and p // 2 + 1 < NPAIR // 2:
                load_text(p // 2 + 1)
            done.append(spectral_front(p, 0))
            if p >= 1:
                spectral_back(p - 1, 0, done[p - 1])
            if g_next < XG:
                if g_next % 2 == 0 and g_next // 2 + 2 < XG // 2:
                    load_group(g_next // 2 + 2)
                mix_group(g_next)
                g_next += 1
        while g_next < XG:
            mix_group(g_next)
            g_next += 1
        spectral_back(NPAIR - 1, 0, done[NPAIR - 1])
        # half 1, software-pipelined by one pair
        done = []
        for p in range(NPAIR):
            done.append(spectral_front(p, 1, act_copies=3))
            if p >= 1:
                spectral_back(p - 1, 1, done[p - 1])
        spectral_back(NPAIR - 1, 1, done[NPAIR - 1])
    if split:
        _split_excess_waits(nc)
    return nc


_CACHE = {}


def _get_program():
    if "nc" not in _CACHE:
        _CACHE["nc"] = _build_program()
    return _CACHE["nc"]


def _host_inputs(x, mixer, transfer, gain):
    D, E1, E2 = _build_matrices()
    g = float(np.asarray(gain).reshape(-1)[0])
    dmat = np.ascontiguousarray(D, dtype=np.float16)
    emat = np.ascontiguousarray(
        np.concatenate([E1 * g, E2 * g], axis=1), dtype=np.float16)

    x = np.asarray(x, dtype=np.float32)
    mixer = np.asarray(mixer, dtype=np.float32)
    transfer = np.asarray(transfer, dtype=np.float32)

    rows = _packed_rows()
    r_of_row = np.array([r for (_, r) in rows], dtype=np.int64)

    in_maps = []
    for core in range(NCORES):
        b, h = divmod(core, 2)
        tr = transfer[CH * h:CH * (h + 1)]           # [32, 257]
        tt = tr[:, r_of_row].T.reshape(4, 128, CH)   # [ri, q, ch]
        text = np.zeros((128, NPAIR, 4, 260), dtype=np.float32)
        text[:, :, :, 1] = 1.0
        text[:, :, :, 131] = 1.0
        for p in range(NPAIR):
            for ri in range(4):
                text[:, p, ri, 2:130] = tt[ri, :, 2 * p][:, None]
                text[:, p, ri, 132:260] = tt[ri, :, 2 * p + 1][:, None]
        in_maps.append({
            "xin": np.ascontiguousarray(x[b], dtype=np.float16),
            "msb": np.ascontiguousarray(mixer[:, CH * h:CH * (h + 1)],
                                        dtype=np.float16),
            "dmat": dmat,
            "emat": emat,
            "text": np.ascontiguousarray(
                text.reshape(128, NPAIR * 1040), dtype=np.float16),
        })
    return in_maps


def _run(in_maps, trace=False):
    nc = _get_program()
    return run_bass_kernel_spmd(nc, in_maps, list(range(NCORES)), trace=trace)


def kernel(x, mixer, transfer, gain, _trace=False):
    in_maps = _host_inputs(x, mixer, transfer, gain)
    res = _run(in_maps, trace=_trace)
    out = np.empty((B, C, T), dtype=np.float32)
    for core in range(NCORES):
        b, h = divmod(core, 2)
        out[b, CH * h:CH * (h + 1), :] = \
            res.results[core]["out"].astype(np.float32).reshape(CH, T)
    if _trace:
        _CACHE["last_result"] = res
    return out


# revision 29
# speedup vs baseline: 1.0498x; 1.0191x over previous
"""Trainium2 Bass kernel for nn_Block_68633577390412.

Pipeline (reference): channel mix (64x64) -> frame into 256 half-overlapping
512-windows -> rfft -> per-(c,k) IIR over frames y_f = (s_f + y_{f-1})*t ->
irfft -> hann window -> overlap-add -> tanh(gain*.)

Sharding: 8 cores = 4 batches x 2 channel-halves, no cross-core traffic.

Per-core design (v2 — hop-block DFT + fp16 datapath):
  - Adjacent frames overlap by half a window (hop 256), so the rfft of frame
    f decomposes into two half-window transforms: S_f = A_f + (-1)^r A_{f+1}
    where A_h[r] = sum_{w<256} x[256h+w] e^{-i 2pi r w/512}. Each hop block's
    A is computed once (K=256 contraction, half the tensor-engine streams of
    the direct K=512 frame DFT); the +/- combine is a single fused
    scalar_tensor_tensor sweep on the vector engine.
  - Packed-real spectral rows are grouped by frequency parity
    (C0=Re-even, C1=Re-odd, C2=[Re_256|Im-even], C3=Im-odd) so (-1)^r is a
    constant sign per 128-row chunk and the combine needs no row masks.
  - The whole spectral datapath runs in fp16 (inputs, DFT bases, scan state
    s/y, transfer t, output): fp16 keeps ~1e-3 relative accuracy (tolerance
    is 2e-2) and unlocks the DVE 2x/4x wide modes plus half-size DMA.
  - The frame recurrence is the DVE TensorTensorScan instruction with fp32
    internal state, reading fp16 s tiles from SBUF.
  - irfft + hann + overlap-add stay fused as two accumulating matmul streams
    per output tile against precomputed E1/E2 = (irfft-basis * hann * gain)
    halves; tanh runs on the scalar engine over both channels at once and
    the output DMA stores fp16 with a (frame, channel, w) interleaved view.
  - All DMA goes through HWDGE (SP-engine issue, ~0.6us shared-device
    overhead) instead of Pool SWDGE (~1us Pool-engine hold each), freeing
    the Pool engine to zero scan-boundary columns.
  - Mix runs as a single fp16 stream (K=64) into PSUM, copied to the
    time-on-partitions fp16 signal buffer by the scalar engine; spectral
    work starts once the first ~17/32 of the signal is mixed and the
    remaining mix groups fill tensor-engine gaps between spectral pairs.
  - TRN2 instructions carry a single sync-wait slot; a post-pass hoists
    excess Tile-assigned waits onto standalone event-semaphore waits.
"""
import numpy as np
import ml_dtypes
from contextlib import ExitStack

import concourse.bass as bass
import concourse.tile as tile
from concourse import mybir
from concourse.bass_utils import run_bass_kernel_spmd

F32 = mybir.dt.float32
F16 = mybir.dt.float16

B, C, T = 4, 64, 65536
WS, STEP, K, NF = 512, 256, 257, 256
NCORES = 8
CH = 32            # channels per core
NCHUNK = 512       # 128-sample time chunks in T
XG = 32            # mix groups (2048 samples each)
NPAIR = CH // 2    # channel pairs per core

SIGMA = [1.0, -1.0, 1.0, -1.0]   # (-1)^r per packed row chunk C0..C3


def _packed_rows():
    """Packed-real row order: (type, r) per packed row, grouped so that
    (-1)^r is constant per 128-row chunk."""
    rows = []
    rows += [("re", r) for r in range(0, 256, 2)]          # C0: Re even
    rows += [("re", r) for r in range(1, 256, 2)]          # C1: Re odd
    rows += [("re", 256)] + [("im", r) for r in range(2, 256, 2)]  # C2
    rows += [("im", r) for r in range(1, 256, 2)]          # C3: Im odd
    assert len(rows) == 512
    return rows


def _build_matrices():
    rows = _packed_rows()
    w = np.arange(WS, dtype=np.float64)
    wh = w[:STEP]
    # D~ [256, 512]: half-window DFT basis, packed/parity-ordered columns.
    D = np.empty((STEP, 512), dtype=np.float64)
    for j, (ty, r) in enumerate(rows):
        if ty == "re":
            D[:, j] = np.cos(2 * np.pi * r * wh / WS)
        else:
            D[:, j] = -np.sin(2 * np.pi * r * wh / WS)
    # G [512, 512]: packed irfft basis rows in the same order.
    G = np.empty((512, WS), dtype=np.float64)
    for j, (ty, r) in enumerate(rows):
        if ty == "re":
            coef = 1.0 / WS if r in (0, 256) else 2.0 / WS
            G[j, :] = coef * np.cos(2 * np.pi * r * w / WS)
        else:
            G[j, :] = -(2.0 / WS) * np.sin(2 * np.pi * r * w / WS)
    hann = 0.5 * (1.0 - np.cos(2.0 * np.pi * w / WS))
    E1 = G[:, :STEP] * hann[None, :STEP]
    E2 = G[:, STEP:] * hann[None, STEP:]
    return D, E1, E2


def _split_excess_waits(nc):
    """TRN2 instructions have a single sync-wait slot. Tile's semaphore
    assignment can attach several waits to one instruction; hoist the extras
    onto standalone event-semaphore wait instructions on the same engine."""
    ctr = 0
    for fn in nc.m.functions:
        for bb in fn.blocks:
            out = []
            changed = False
            for inst in bb.instructions:
                si = inst.sync_info
                ow = list(si.on_wait) if si and si.on_wait else []
                if len(ow) > 1:
                    for wv in ow[:-1]:
                        ev = mybir.InstEventSemaphore(
                            name=f"WSPLIT-{ctr}", ins=[], outs=[])
                        ctr += 1
                        ev.engine = inst.engine
                        evsi = ev.sync_info
                        if evsi is None:
                            ev.sync_info = mybir.SyncInfo(on_wait=[wv],
                                                          on_update=[])
                        else:
                            evsi.on_wait = [wv]
                        out.append(ev)
                    si.on_wait = [ow[-1]]
                    changed = True
                out.append(inst)
            if changed:
                bb.instructions = out
    return ctr


def _build_program(split=True):
    nc = bass.Bass("TRN2", target_bir_lowering=False, debug=False,
                   num_devices=NCORES)
    xin_t = nc.dram_tensor("xin", [C, T], F16, kind="ExternalInput")
    msb_t = nc.dram_tensor("msb", [C, CH], F16, kind="ExternalInput")
    dmat_t = nc.dram_tensor("dmat", [STEP, 512], F16, kind="ExternalInput")
    emat_t = nc.dram_tensor("emat", [512, 512], F16, kind="ExternalInput")
    tpk_t = nc.dram_tensor("tpk", [512, CH], F16, kind="ExternalInput")
    out_t = nc.dram_tensor("out", [CH, 256, 256], F16, kind="ExternalOutput")

    xin = xin_t.ap()
    msb, dmat, emat, tpk = msb_t.ap(), dmat_t.ap(), emat_t.ap(), tpk_t.ap()

    with tile.TileContext(nc) as tc, ExitStack() as ctx:
        cpool = ctx.enter_context(tc.tile_pool(name="const", bufs=1))
        m_sb = cpool.tile([C, CH], F16, tag="m_sb")
        dm_sb = cpool.tile([128, 1024], F16, tag="dm_sb")
        em_sb = cpool.tile([128, 2048], F16, tag="em_sb")
        tp_sb = cpool.tile([128, 4 * CH], F16, tag="tp_sb")

        # x'' : mixed signal, time-on-partitions. col = 32*chunk + d_local.
        xsq = cpool.tile([128, CH * (NCHUNK + 2)], F16, tag="xsq")
        nc.vector.memset(xsq[:, CH * NCHUNK:], 0.0)

        xpool = ctx.enter_context(tc.tile_pool(name="xin", bufs=4))

        def load_consts():
            # issued after the first x tiles: nothing here is needed until
            # the first DFT ~12us in, and HWDGE issue order gates the input.
            for i in range(2):
                nc.sync.dma_start(dm_sb[:, 512 * i:512 * (i + 1)],
                                  dmat[128 * i:128 * (i + 1), :])
            for i in range(4):
                nc.sync.dma_start(tp_sb[:, CH * i:CH * (i + 1)],
                                  tpk[128 * i:128 * (i + 1), :])
            for i in range(4):
                nc.sync.dma_start(em_sb[:, 512 * i:512 * (i + 1)],
                                  emat[128 * i:128 * (i + 1), :])
        mixpool = ctx.enter_context(tc.tile_pool(name="mixps", bufs=2,
                                                 space="PSUM"))
        spool = ctx.enter_context(tc.tile_pool(name="aps", bufs=4,
                                               space="PSUM"))
        opool = ctx.enter_context(tc.tile_pool(name="ops", bufs=2,
                                               space="PSUM"))
        apool = ctx.enter_context(tc.tile_pool(name="asb", bufs=8))
        sbpool = ctx.enter_context(tc.tile_pool(name="ssb", bufs=8))
        ypool = ctx.enter_context(tc.tile_pool(name="y", bufs=16))
        outpool = ctx.enter_context(tc.tile_pool(name="outs", bufs=4))
        # scan carry between frame halves; col = 8p + 2*ri + c01
        carry = cpool.tile([128, 8 * NPAIR], F16, tag="carry")

        xtiles = [None] * (XG // 2)

        def load_group(gt):
            xt = xpool.tile([C, 4096], F16, tag="xt")
            nc.sync.dma_start(xt[:], xin[:, 4096 * gt:4096 * (gt + 1)])
            xtiles[gt] = xt

        def mix_group(g):
            xt = xtiles[g // 2]
            base = 2048 * (g % 2)
            mps = mixpool.tile([128, 512], F32, tag="mps")
            for m in range(16):
                lhsT = xt[:, base + 128 * m:base + 128 * (m + 1)]
                nc.tensor.matmul(mps[:, CH * m:CH * (m + 1)], lhsT, m_sb[:],
                                 start=True, stop=True)
            nc.scalar.copy(xsq[:, 512 * g:512 * (g + 1)], mps[:])

        def spectral_front(p, half, act_copies=2):
            b0 = 128 * half            # first hop block of this half
            # A[r, (c01, blk)] for 129 blocks; one matmul per (ri, wi).
            s_tiles = []
            for ri in range(4):
                aps = spool.tile([128, 258], F32, tag="aps")
                for wi in range(2):
                    lhsT = dm_sb[:, 512 * wi + 128 * ri:
                                 512 * wi + 128 * (ri + 1)]
                    v = xsq[:, CH * (2 * b0 + wi) + 2 * p:
                            CH * (2 * b0 + wi) + 2 * p + 2]
                    rhs = bass.AP(v.tensor, v.offset,
                                  [list(q) for q in v.ap] + [[2 * CH, 129]])
                    nc.tensor.matmul(aps[:], lhsT, rhs,
                                     start=(wi == 0), stop=(wi == 1))
                # PSUM exit (only one PSUM src allowed per vector op): copy A
                # to fp16 SBUF, split between Act and DVE to balance engines.
                a_sb = apool.tile([128, 258], F16, tag="a")
                if ri < act_copies:
                    nc.scalar.copy(a_sb[:], aps[:])
                else:
                    nc.vector.tensor_copy(a_sb[:], aps[:])
                # s[:, c01*128+f] = A[:, c01*129+f] +/- A[:, c01*129+f+1];
                # plain TensorTensor gets the DVE 2x fp16 mode.
                s_sb = sbpool.tile([128, 256], F16, tag="s")
                in0 = bass.AP(a_sb.tensor, a_sb.offset,
                              [list(a_sb.ap[0]), [129, 2], [1, 128]])
                in1 = bass.AP(a_sb.tensor, a_sb.offset + 1,
                              [list(a_sb.ap[0]), [129, 2], [1, 128]])
                outv = bass.AP(s_sb.tensor, s_sb.offset,
                               [list(s_sb.ap[0]), [128, 2], [1, 128]])
                op = (mybir.AluOpType.add if SIGMA[ri] > 0
                      else mybir.AluOpType.subtract)
                # the scan ISA only exists on DVE; give Pool most of the
                # combines instead (DVE is the co-bottleneck with PE)
                n_pool = 3 if half == 0 else 2
                tt_eng = nc.gpsimd if ri < n_pool else nc.vector
                tt_eng.tensor_tensor(outv, in0, in1, op)
                s_tiles.append(s_sb)

            y_tiles = {}
            for c01 in range(2):
                for ri in range(4):
                    yt = ypool.tile([128, 129], F16, tag="y")
                    cidx = p * 8 + c01 * 4 + ri
                    t_col = tp_sb[:, CH * ri + 2 * p + c01:
                                  CH * ri + 2 * p + c01 + 1]
                    if half == 0:
                        nc.vector.memset(yt[:, 0:1], 0.0)
                        initial = 0.0
                    else:
                        nc.vector.tensor_copy(yt[:, 0:1],
                                              carry[:, cidx:cidx + 1])
                        initial = carry[:, cidx:cidx + 1]
                    nc.vector.tensor_tensor_scan(
                        yt[:, 1:129],
                        s_tiles[ri][:, 128 * c01:128 * (c01 + 1)],
                        t_col.broadcast_to((128, 128)),
                        initial, mybir.AluOpType.add, mybir.AluOpType.mult)
                    if half == 0:
                        nc.vector.tensor_copy(carry[:, cidx:cidx + 1],
                                              yt[:, 128:129])
                    y_tiles[(c01, ri)] = yt
            return y_tiles

        def spectral_back(p, half, y_tiles):
            # iDFT + hann + OLA; per-channel PSUM tiles so the tensor engine
            # is not gated on the other channel's tanh.
            oa = out_t.ap()
            for c01 in range(2):
                sl = opool.tile([128, 256], F32, tag="ops")
                for ri in range(4):
                    yt = y_tiles[(c01, ri)]
                    nc.tensor.matmul(sl[:], yt[:, 1:129],
                                     em_sb[:, 512 * ri:512 * ri + 256],
                                     start=(ri == 0), stop=False)
                    nc.tensor.matmul(sl[:], yt[:, 0:128],
                                     em_sb[:, 512 * ri + 256:512 * (ri + 1)],
                                     start=False, stop=(ri == 3))
                ot = outpool.tile([128, 256], F16, tag="ot")
                nc.scalar.activation(ot[:], sl[:],
                                     mybir.ActivationFunctionType.Tanh)
                nc.sync.dma_start(
                    oa[2 * p + c01, 128 * half:128 * (half + 1), :], ot[:])

        GSPLIT = 17          # mix groups needed before frame-half 0
        nc.sync.dma_start(m_sb[:], msb[:])
        for gt in range(2):
            load_group(gt)
        load_consts()
        for g in range(GSPLIT):
            if g % 2 == 0 and g // 2 + 2 < XG // 2:
                load_group(g // 2 + 2)
            mix_group(g)
        g_next = GSPLIT
        # half 0, software-pipelined by two pairs (the merged scan's latency
        # spans more than one tensor-engine stage); mix groups fill PE gaps
        done = []
        for p in range(NPAIR):
            done.append(spectral_front(p, 0))
            if p >= 1:
                spectral_back(p - 1, 0, done[p - 1])
            if g_next < XG:
                if g_next % 2 == 0 and g_next // 2 + 2 < XG // 2:
                    load_group(g_next // 2 + 2)
                mix_group(g_next)
                g_next += 1
        while g_next < XG:
            mix_group(g_next)
            g_next += 1
        spectral_back(NPAIR - 1, 0, done[NPAIR - 1])
        # half 1, software-pipelined by one pair
        done = []
        for p in range(NPAIR):
            done.append(spectral_front(p, 1, act_copies=3))
            if p >= 1:
                spectral_back(p - 1, 1, done[p - 1])
        spectral_back(NPAIR - 1, 1, done[NPAIR - 1])
    if split:
        _split_excess_waits(nc)
    return nc


_CACHE = {}


def _get_program():
    if "nc" not in _CACHE:
        _CACHE["nc"] = _build_program()
    return _CACHE["nc"]


def _host_inputs(x, mixer, transfer, gain):
    D, E1, E2 = _build_matrices()
    g = float(np.asarray(gain).reshape(-1)[0])
    dmat = np.ascontiguousarray(D, dtype=np.float16)
    emat = np.ascontiguousarray(
        np.concatenate([E1 * g, E2 * g], axis=1), dtype=np.float16)

    x = np.asarray(x, dtype=np.float32)
    mixer = np.asarray(mixer, dtype=np.float32)
    transfer = np.asarray(transfer, dtype=np.float32)

    rows = _packed_rows()
    r_of_row = np.array([r for (_, r) in rows], dtype=np.int64)

    in_maps = []
    for core in range(NCORES):
        b, h = divmod(core, 2)
        tr = transfer[CH * h:CH * (h + 1)]           # [32, 257]
        tpk = np.ascontiguousarray(tr[:, r_of_row].T,
                                   dtype=np.float16)  # [512, 32]
        in_maps.append({
            "xin": np.ascontiguousarray(x[b], dtype=np.float16),
            "msb": np.ascontiguousarray(mixer[:, CH * h:CH * (h + 1)],
                                        dtype=np.float16),
            "dmat": dmat,
            "emat": emat,
            "tpk": tpk,
        })
    return in_maps


def _run(in_maps, trace=False):
    nc = _get_program()
    return run_bass_kernel_spmd(nc, in_maps, list(range(NCORES)), trace=trace)


def kernel(x, mixer, transfer, gain, _trace=False):
    in_maps = _host_inputs(x, mixer, transfer, gain)
    res = _run(in_maps, trace=_trace)
    out = np.empty((B, C, T), dtype=np.float32)
    for core in range(NCORES):
        b, h = divmod(core, 2)
        out[b, CH * h:CH * (h + 1), :] = \
            res.results[core]["out"].astype(np.float32).reshape(CH, T)
    if _trace:
        _CACHE["last_result"] = res
    return out
